# revision 15
# baseline (speedup 1.0000x reference)
"""SNN leaky integrate-and-fire kernel for Trainium2 (8 NeuronCores, SPMD).

Computes, for x [30, 8192, 784] f32 and W [10, 784] f32:
    w_q  = 16-bit fixed-point quantized W (Q3.12, straight-through)
    cur  = einsum('tbi,oi->tbo', x, w_q)                  [30, 8192, 10]
    scan over t: mem_t = BETA*mem_{t-1} + cur_t - spk_{t-1}; spk_t = mem_t > 1
Returns (spk_rec, mem_rec), each [30, 8192, 10] f32.

Sharding: pure data parallel over the batch axis (1024 rows per core).
Per core the input is pre-transposed on host to xt [784, 30*1024] so the
contraction axis (784) lies on SBUF partitions with fully contiguous DMA.

The matmul uses 4x column-group tiling of the PE array: the per-core batch
is split into 4 quarters of 256; quarter j's outputs live on partitions
32j + 3o (o = 0..9; weights are zero-padded to M=32 with values at weight
columns 3o, so the in-between partitions hold clean zeros and the output
partitions spread across 14 of the 16 DMA engines).  Four matmuls (one per
column group) run concurrently, which quadruples fp32 matmul throughput.
The membrane scan runs directly on the [128, 256] layout with states
written straight into the output staging buffers; spikes are staged and
written out as uint8 (4x fewer bytes), converted back to f32 on host.
"""

import numpy as np

import concourse.bass as bass
import concourse.mybir as mybir
from concourse import bacc
import concourse.tile as tile
from concourse.bass_utils import run_bass_kernel_spmd

N_CORES = 8
T = 30
B = 8192
I = 784
O = 10
BC = B // N_CORES          # 1024 batch rows per core
BQ = BC // 4               # 256 batch rows per column group
N = T * BC                 # 30720 columns per core (t-major, then b)
I_MAIN = 768               # 6 contraction chunks of 128
I_TAIL = I - I_MAIN        # 16
BETA = 0.9375
THRESHOLD = 1.0

f32 = mybir.dt.float32
u8 = mybir.dt.uint8


def _build_nc():
    AL = mybir.AluOpType
    nc = bacc.Bacc("TRN2")
    xt = nc.dram_tensor("xt", [I, N], f32, kind="ExternalInput")
    wt = nc.dram_tensor("wt", [I, O], f32, kind="ExternalInput")
    spk = nc.dram_tensor("spk", [4, O, T, BQ], u8, kind="ExternalOutput")
    mem = nc.dram_tensor("mem", [4, O, T, BQ], f32, kind="ExternalOutput")

    def out_rows(stage, j, width):
        # partitions 32j + 3o + 2 for o in 0..9, as a [10, width] AP
        return stage[32 * j + 2:32 * j + 32, 0:width].rearrange(
            "(o s) q -> o s q", s=3)[:, 0, :]

    with tile.TileContext(nc) as tc:
        with (
            tc.tile_pool(name="xmain", bufs=3) as xmain_pool,
            tc.tile_pool(name="xtail", bufs=4) as xtail_pool,
            tc.tile_pool(name="stage", bufs=2) as stage_pool,
            tc.tile_pool(name="state", bufs=1) as state_pool,
            tc.tile_pool(name="psum", bufs=8, space="PSUM") as psum_pool,
        ):
            # Stationary weights, zero-padded to M=32 with the 10 real rows
            # at weight columns 3o.  The 16-row tail chunk (c=6) is kept at
            # two partition offsets (0 and 64) so its x transfers alternate
            # between the two DMA engine groups.
            wt_sb = state_pool.tile([128, 7, 32], f32)
            nc.vector.memset(wt_sb, 0.0)
            for c in range(6):
                nc.gpsimd.dma_start(
                    out=wt_sb[:, c, 2:32].rearrange("p (o s) -> p o s", s=3)[:, :, 0],
                    in_=wt[c * 128:(c + 1) * 128, :])
            for row in (0, 64):
                nc.gpsimd.dma_start(
                    out=wt_sb[row:row + I_TAIL, 6, 2:32].rearrange(
                        "p (o s) -> p o s", s=3)[:, :, 0],
                    in_=wt[I_MAIN:, :])

            spk_zero = state_pool.tile([128, BQ], u8)
            nc.vector.memset(spk_zero, 0)
            mem_zero = state_pool.tile([128, BQ], f32)
            nc.vector.memset(mem_zero, 0.0)
            spk_prev = spk_zero
            mem_prev = mem_zero

            xt_main = xt[0:I_MAIN, :].rearrange("(c p) n -> p c n", p=128)

            # x tiles: 2 timesteps each (matmuls then run N=512 over the
            # (2, 256) free pattern, halving instruction count); the last
            # four timesteps use 1-t tiles with per-chunk DMA splits so the
            # tensor engine drains right behind the final loads.
            tile_ts = [2] * 14 + [1, 1]
            # output flush groups: mostly 5 timesteps, tiny at the end.
            flush_ts = [5, 5, 5, 5, 5, 4, 1]
            assert sum(tile_ts) == T and sum(flush_ts) == T

            t = 0
            fg = 0                 # flush group index
            fg_done = 0            # timesteps completed in current group
            kt = flush_ts[0]
            spk_stage = mem_stage = None
            xtl = None
            row = 0
            for w, nts in enumerate(tile_ts):
                xm = xmain_pool.tile([128, 6, 2, BC], f32, tag="xm")
                if nts == 2 and w < 4:
                    # split the first tiles across both HWDGE rings so the
                    # DMA pipe fills faster at kernel start
                    src_v = xt_main[:, :, t * BC:(t + 2) * BC].rearrange(
                        "p c (s n) -> p c s n", s=2)
                    nc.sync.dma_start(out=xm[:, 0:3], in_=src_v[:, 0:3])
                    nc.scalar.dma_start(out=xm[:, 3:6], in_=src_v[:, 3:6])
                elif nts == 2:
                    nc.sync.dma_start(
                        out=xm,
                        in_=xt_main[:, :, t * BC:(t + 2) * BC].rearrange(
                            "p c (s n) -> p c s n", s=2))
                else:
                    for c in range(6):
                        nc.sync.dma_start(
                            out=xm[:, c, 0, :],
                            in_=xt_main[:, c, t * BC:(t + 1) * BC])
                # 16-row tail chunk: two timesteps per tile; partition
                # offset alternates 0/64 per tile to balance the narrow
                # transfers over both DMA engine groups.  The final two
                # timesteps load singly (one per offset) so both groups
                # carry exactly 7.5 tiles.
                if t >= T - 2:
                    row = 64 * (t % 2)
                    xtl = xtail_pool.tile([128, 2, BC], f32, tag="xtl")
                    nc.sync.dma_start(
                        out=xtl[row:row + I_TAIL, 0, :],
                        in_=xt[I_MAIN:, t * BC:(t + 1) * BC])
                elif t % 2 == 0:
                    row = 64 * ((t // 2) % 2)
                    xtl = xtail_pool.tile([128, 2, BC], f32, tag="xtl")
                    tail_ring = nc.gpsimd if w < 4 else nc.sync
                    tail_ring.dma_start(
                        out=xtl[row:row + I_TAIL, :, :],
                        in_=xt[I_MAIN:, t * BC:(t + 2) * BC].rearrange(
                            "p (s n) -> p s n", s=2))

                ps = psum_pool.tile([128, 2, BQ], f32)
                for c in range(6):
                    lhsT = wt_sb[:, c, :]
                    for j in range(4):
                        nc.tensor.matmul(
                            ps[32 * j:32 * j + 32, 0:nts, :],
                            lhsT=lhsT,
                            rhs=xm[:, c, 0:nts, j * BQ:(j + 1) * BQ],
                            start=(c == 0),
                            stop=False,
                            tile_position=(0, 32 * j),
                        )
                ttl0 = 0 if (nts == 2 or t >= T - 2) else (t % 2)
                for j in range(4):
                    nc.tensor.matmul(
                        ps[32 * j:32 * j + 32, 0:nts, :],
                        lhsT=wt_sb[row:row + I_TAIL, 6, :],
                        rhs=xtl[row:row + I_TAIL, ttl0:ttl0 + nts, j * BQ:(j + 1) * BQ],
                        start=False,
                        stop=True,
                        tile_position=(row, 32 * j),
                    )

                for tt in range(nts):
                    if fg_done == 0:
                        kt = flush_ts[fg]
                        spk_stage = stage_pool.tile([128, 5 * BQ], u8, tag="spk")
                        mem_stage = stage_pool.tile([128, 5 * BQ], f32, tag="mem")
                    off = fg_done * BQ
                    spk_sl = spk_stage[:, off:off + BQ]
                    mem_sl = mem_stage[:, off:off + BQ]

                    # mem_t = (mem_{t-1} * BETA) + cur_t  (ref op order)
                    nc.vector.scalar_tensor_tensor(
                        out=mem_sl, in0=mem_prev, scalar=BETA, in1=ps[:, tt, :],
                        op0=AL.mult, op1=AL.add,
                    )
                    # mem_t -= spk_{t-1} * THRESHOLD (THRESHOLD == 1)
                    nc.vector.tensor_sub(mem_sl, mem_sl, spk_prev)
                    nc.vector.tensor_scalar(
                        out=spk_sl, in0=mem_sl, scalar1=THRESHOLD,
                        scalar2=None, op0=AL.is_gt,
                    )
                    spk_prev = spk_sl
                    mem_prev = mem_sl

                    fg_done += 1
                    if fg_done == kt:
                        t0 = t + tt - (kt - 1)
                        # final group rides the (idle-by-then) sync ring for
                        # mem so the two flushes run in parallel.
                        mem_ring = nc.sync if fg == len(flush_ts) - 1 else nc.scalar
                        for j in range(4):
                            nc.scalar.dma_start(
                                out=spk[j, :, t0:t0 + kt, :],
                                in_=out_rows(spk_stage, j, kt * BQ).rearrange(
                                    "o (k q) -> o k q", k=kt),
                            )
                            mem_ring.dma_start(
                                out=mem[j, :, t0:t0 + kt, :],
                                in_=out_rows(mem_stage, j, kt * BQ).rearrange(
                                    "o (k q) -> o k q", k=kt),
                            )
                        fg += 1
                        fg_done = 0
                t += nts

    nc.finalize()
    return nc


_NC = None


def _get_nc():
    global _NC
    if _NC is None:
        _NC = _build_nc()
    return _NC


def _quantize_w(W):
    W32 = np.asarray(W, dtype=np.float32)
    q = np.round(W32 * np.float32(4096.0))
    q = np.clip(q, np.float32(-32768.0), np.float32(32767.0)) / np.float32(4096.0)
    # straight-through forward value, replicated bit-exactly: w + (q - w)
    return (W32 + (q - W32)).astype(np.float32)


def kernel(x, W, _run_opts=None):
    x = np.asarray(x, dtype=np.float32)
    W = np.asarray(W, dtype=np.float32)
    assert x.shape == (T, B, I) and W.shape == (O, I)

    wt = np.ascontiguousarray(_quantize_w(W).T)  # [784, 10]

    in_maps = []
    for c in range(N_CORES):
        xc = x[:, c * BC:(c + 1) * BC, :]                      # [30, 1024, 784]
        xt_c = np.ascontiguousarray(xc.transpose(2, 0, 1))     # [784, 30, 1024]
        in_maps.append({"xt": xt_c.reshape(I, N), "wt": wt})

    nc = _get_nc()
    run_opts = dict(_run_opts or {})
    res = run_bass_kernel_spmd(nc, in_maps, core_ids=list(range(N_CORES)), **run_opts)

    spk_full = np.empty((T, B, O), dtype=np.float32)
    mem_full = np.empty((T, B, O), dtype=np.float32)
    for c in range(N_CORES):
        # device layout [4, 10, T, 256] -> [T, 1024, 10]
        s = res.results[c]["spk"].astype(np.float32).transpose(2, 0, 3, 1).reshape(T, BC, O)
        m = res.results[c]["mem"].transpose(2, 0, 3, 1).reshape(T, BC, O)
        spk_full[:, c * BC:(c + 1) * BC, :] = s
        mem_full[:, c * BC:(c + 1) * BC, :] = m

    if _run_opts is not None:
        kernel.last_result = res
    return spk_full, mem_full
